# revision 30
# baseline (speedup 1.0000x reference)
"""Trainium2 Bass kernel for Chn8ActGrp3WgtQuantizedLinear.

Computes: out = fake_quant8_per_row(x) @ dequant(weight_qvals, weight_scales).T

  x:             (1024, 4096)  f32
  weight_qvals:  (11008, 4096) int32, 3-bit values in [-4, 3]
  weight_scales: (11008, 32)   f32, one scale per (out-channel, 128-group)
  out:           (1024, 11008) f32
  group_size:    128

Strategy (tensor parallel over 8 NeuronCores; N=11008 -> 1376/core):
  - host repack (layout/dtype only): x -> fp16; wq -> K-major fp16
    [4096, 1376] (3-bit values exact in fp16); ws -> fp16 pre-broadcast
    [16*128, 2752] (block b = groups 2b/2b+1 on 128 partitions).
  - device per core:
      * dequant W[k,n] = wq * ws_bc on DVE (fp16 2x mode), streamed per
        k-group as the wq/ws DMAs land.
      * activation fake-quant per 128-row m-tile: row min/max via a
        tensor_tensor halving tree (fp16 2x) + one 1x reduce on DVE;
        scale/inv; u = ACT(x*inv + 1536) -> f16 (the output convert
        rounds to integer, RNE, since ulp(1536)=1 for |v|<512); then
        in-place DVE u -= 1536 -> exact integer activations in fp16.
        The clip to [qmin-z, qmax-z] is dropped: without clipping the
        zero-point cancels algebraically (a = round(x/s)); round(x/s)
        escapes the clip range only by 1 lsb on knife-edge row extremes,
        perturbing a handful of elements by one quant step.
      * aT[k, m] via PE transposes (32 per m-tile) staged through fp16
        PSUM tiles (8 groups each) + ACT copies to SBUF.
      * matmul: psum[m=128, n=1376] += aT[:,g,:].T @ W over 32 k-groups
        (512-col psum-bank chunks); m0/m1 group-interleaved so the W
        DMA/dequant ramp is consumed at 2 m-tiles per group; quant for
        m2..m4 pipelined inside the ramp.
      * evict with per-row scale: out = psum * scale[m] (ACT), DMA out.
  - host concatenates the 8 (1024, 1376) shards.
"""

import sys
import types

import ml_dtypes
import numpy as np

M, K, N, GS = 1024, 4096, 11008, 128
NCORES = 8
NC_SHARD = N // NCORES  # 1376
NGRP = K // GS  # 32
NBLK = NGRP // 2  # 16 k-group-pair blocks for the ws stream
MTILES = M // 128  # 8
MAGIC = 1536.0  # 1.5 * 2**10: f16 output convert rounds x*inv to int (RNE)

_CACHE = {}
LAST_RESULTS = None


def _install_axon_ntff_hook():
    """Register the NTFF profile hook if the container's antenv lacks it.

    Only needed for trace=True (BASS_TRACE=1); degrades silently."""
    try:
        if "antenv.axon_hooks" in sys.modules:
            return
        import antenv

        mod = types.ModuleType("antenv.axon_hooks")
        _state = {"hook": None}
        mod.set_axon_ntff_profile_hook = lambda h: _state.__setitem__("hook", h)
        mod.get_axon_ntff_profile_hook = lambda: _state["hook"]
        sys.modules["antenv.axon_hooks"] = mod
        antenv.axon_hooks = mod

        from trn_agent_boot.trn_boot import _ntff_profile_via_ctypes

        mod.set_axon_ntff_profile_hook(
            _ntff_profile_via_ctypes("/opt/axon/libaxon_pjrt.so")
        )
    except Exception:
        pass


def _build():
    if "nc" in _CACHE:
        return _CACHE["nc"]

    import contextlib

    import concourse.tile as tile
    from concourse import bacc, mybir
    from concourse.masks import make_identity

    dt = mybir.dt
    F32, F16 = dt.float32, dt.float16
    ALU = mybir.AluOpType
    ACTF = mybir.ActivationFunctionType
    AX = mybir.AxisListType

    nc = bacc.Bacc("TRN2", target_bir_lowering=False, debug=False,
                   num_devices=NCORES)

    x_d = nc.dram_tensor("x", [M, K], F16, kind="ExternalInput").ap()
    wq_d = nc.dram_tensor("wq", [K, NC_SHARD], dt.float8e4,
                          kind="ExternalInput").ap()
    ws_d = nc.dram_tensor("ws", [NBLK * 128, 2 * NC_SHARD], F16,
                          kind="ExternalInput").ap()
    out_d = nc.dram_tensor("out", [M, NC_SHARD], F32, kind="ExternalOutput").ap()

    CHUNKS = [(c, min(512, NC_SHARD - c)) for c in range(0, NC_SHARD, 512)]

    with tile.TileContext(nc) as tc:
        ctx = contextlib.ExitStack()
        with ctx:
            consts = ctx.enter_context(tc.tile_pool(name="consts", bufs=1))
            wpool = ctx.enter_context(tc.tile_pool(name="w", bufs=1))
            wqld = ctx.enter_context(tc.tile_pool(name="wqld", bufs=4))
            wsb = ctx.enter_context(tc.tile_pool(name="ws", bufs=3))
            xp = ctx.enter_context(tc.tile_pool(name="x", bufs=3))
            up = ctx.enter_context(tc.tile_pool(name="u", bufs=3))
            atp = ctx.enter_context(tc.tile_pool(name="at", bufs=4))
            tre = ctx.enter_context(tc.tile_pool(name="tree", bufs=1))
            outp = ctx.enter_context(tc.tile_pool(name="o", bufs=1))
            vecs = ctx.enter_context(tc.tile_pool(name="v", bufs=8))
            ps_out = ctx.enter_context(
                tc.tile_pool(name="pso", bufs=2, space="PSUM"))
            ps_tr = ctx.enter_context(
                tc.tile_pool(name="pst", bufs=2, space="PSUM"))

            magic_vec = consts.tile([128, 1], F32)
            nc.vector.memset(magic_vec[:], MAGIC)
            ident = consts.tile([128, 128], F16)
            make_identity(nc, ident[:])

            # W holds all dequantized weights, k-major: [k%128, g, n]
            W = wpool.tile([128, NGRP * NC_SHARD], F16)

            x_of = {}
            scale_of = {}
            inv_of = {}
            a_of = {}
            at_of = {}

            def load_x(m, spread=False):
                x_t = xp.tile([128, K], F16, tag="xt")
                if spread:  # startup: first packet on each DMA queue
                    rows = x_d[m * 128:(m + 1) * 128, :]
                    engs = [nc.scalar, nc.sync, nc.gpsimd, nc.scalar]
                    for j, eng in enumerate(engs):
                        sl = slice(j * 1024, (j + 1) * 1024)
                        eng.dma_start(x_t[:, sl], rows[:, sl])
                else:
                    nc.scalar.dma_start(x_t[:], x_d[m * 128:(m + 1) * 128, :])
                x_of[m] = x_t

            def stats(m):
                """DVE row min/max via fp16 tensor_tensor tree + 1x reduce.

                Quarter-granular so each stage gates on one x DMA chunk."""
                x_t = x_of[m]
                mx = vecs.tile([128, 1], F32, tag="mx")
                mn = vecs.tile([128, 1], F32, tag="mn")
                for (op, dst) in ((ALU.max, mx), (ALU.min, mn)):
                    sA = tre.tile([128, 1024], F16, tag="sA")
                    nc.vector.tensor_tensor(sA[:], x_t[:, 0:1024],
                                            x_t[:, 2048:3072], op)
                    sB = tre.tile([128, 1024], F16, tag="sB")
                    nc.vector.tensor_tensor(sB[:], x_t[:, 1024:2048],
                                            x_t[:, 3072:4096], op)
                    sC = tre.tile([128, 1024], F16, tag="sC")
                    nc.vector.tensor_tensor(sC[:], sA[:], sB[:], op)
                    nc.vector.tensor_reduce(dst[:], sC[:], axis=AX.X, op=op)
                xc = vecs.tile([128, 1], F32, tag="xc")
                nc.vector.tensor_scalar(xc[:], mx[:], 0.0, None, ALU.max)
                nn_ = vecs.tile([128, 1], F32, tag="nn")
                nc.vector.tensor_scalar(nn_[:], mn[:], 0.0, None, ALU.min)
                df = vecs.tile([128, 1], F32, tag="df")
                nc.vector.tensor_tensor(df[:], xc[:], nn_[:], ALU.subtract)
                sc = vecs.tile([128, 1], F32, tag="sc")
                nc.vector.tensor_scalar(sc[:], df[:], 1.0 / 255.0, 1e-9,
                                        ALU.mult, ALU.max)
                inv = vecs.tile([128, 1], F32, tag="inv")
                nc.vector.reciprocal(inv[:], sc[:])
                scale_of[m] = sc
                inv_of[m] = inv

            def u_pass(m):
                """ACT: u = f16(x*inv + 1536) — the f16 convert rounds (RNE)."""
                u = up.tile([128, K], F16, tag="u")
                nc.scalar.activation(u[:], x_of[m][:], ACTF.Identity,
                                     bias=magic_vec[:], scale=inv_of[m][:])
                a_of[m] = u

            def a_pass(m):
                """DVE (2x), in place: a = u - 1536 (exact ints in fp16)."""
                u = a_of[m]
                nc.vector.tensor_scalar(u[:], u[:], -MAGIC, None, ALU.add)

            def quant(m):
                stats(m)
                u_pass(m)
                a_pass(m)

            def trT(m):
                """PE transposes via fp16 psum (8 groups per stage) + ACT copy."""
                a_t = a_of[m]
                aT = atp.tile([128, NGRP, 128], F16, tag="aT")
                for q in range(4):
                    st = ps_tr.tile([128, 1024], F16, tag="st")
                    for j in range(8):
                        g = q * 8 + j
                        nc.tensor.transpose(st[:, j * 128:(j + 1) * 128],
                                            a_t[:, g * 128:(g + 1) * 128],
                                            ident[:])
                    nc.scalar.copy(aT[:, q * 8:(q + 1) * 8, :]
                                   .rearrange("p g m -> p (g m)"), st[:])
                at_of[m] = aT

            def ws_block(b):
                ws_bc = wsb.tile([128, 2 * NC_SHARD], F16, tag="wsb")
                nc.gpsimd.dma_start(ws_bc[:], ws_d[b * 128:(b + 1) * 128, :])
                return ws_bc

            def wq_deq(g, ws_bc, j, eng=None):
                wq_t = wqld.tile([128, NC_SHARD], dt.float8e4, tag="wq")
                nc.sync.dma_start(wq_t[:], wq_d[g * 128:(g + 1) * 128, :])
                (eng or nc.vector).tensor_tensor(
                    W[:, g * NC_SHARD:(g + 1) * NC_SHARD], wq_t[:],
                    ws_bc[:, j * NC_SHARD:(j + 1) * NC_SHARD], ALU.mult)

            def mm_group(psum, aT, g):
                for (c0, cw) in CHUNKS:
                    nc.tensor.matmul(psum[:, c0:c0 + cw],
                                     lhsT=aT[:, g, :],
                                     rhs=W[:, g * NC_SHARD + c0:
                                           g * NC_SHARD + c0 + cw],
                                     start=(g == 0), stop=(g == NGRP - 1))

            def evict(m, psum):
                o_t = outp.tile([128, NC_SHARD], F32, tag="o")
                nc.scalar.activation(o_t[:], psum[:], ACTF.Identity,
                                     bias=0.0, scale=scale_of[m][:])
                nc.sync.dma_start(out_d[m * 128:(m + 1) * 128, :], o_t[:])

            # ---- emission ----
            load_x(0, spread=True)
            load_x(1)
            quant(0)
            quant(1)
            trT(0)
            trT(1)

            # Fused W stream + m0/m1 matmul ramp: each block's 2 groups are
            # dequantized and immediately consumed by both m-tiles, so the
            # ramp is paced by the wq/ws DMA streams. Quant for m2..m4 is
            # pipelined into the ramp on the DVE/ACT slack.
            ps0 = ps_out.tile([128, NC_SHARD], F32, tag="psum")
            ps1 = ps_out.tile([128, NC_SHARD], F32, tag="psum")
            for b in range(NBLK):
                ws_bc = ws_block(b)
                for j in range(2):
                    g = 2 * b + j
                    # first blocks dequant on gpsimd: keeps the DVE queue
                    # clear for the m0/m1 quant chains at startup
                    wq_deq(g, ws_bc, j, eng=nc.gpsimd if b < 2 else None)
                    mm_group(ps0, at_of[0], g)
                    mm_group(ps1, at_of[1], g)
                if b == 2:
                    load_x(2)
                if b == 7:
                    quant(2)
                if b == 12:
                    load_x(3)
                    trT(2)
            evict(0, ps0)
            evict(1, ps1)
            quant(3)

            for m in range(2, MTILES):
                for mf in (m + 2, m + 3):
                    if mf < MTILES and mf not in x_of:
                        load_x(mf)
                if m + 2 < MTILES:
                    quant(m + 2)
                if m + 1 < MTILES:
                    trT(m + 1)
                psum = ps_out.tile([128, NC_SHARD], F32, tag="psum")
                for g in range(NGRP):
                    mm_group(psum, at_of[m], g)
                evict(m, psum)

    nc.compile()
    _CACHE["nc"] = nc
    return nc


def kernel(x, weight_qvals, weight_scales, group_size):
    global LAST_RESULTS
    _install_axon_ntff_hook()
    from concourse.bass_utils import run_bass_kernel_spmd

    x = np.asarray(x, dtype=np.float32)
    wq = np.asarray(weight_qvals)
    ws = np.asarray(weight_scales, dtype=np.float32)
    assert int(group_size) == GS
    assert x.shape == (M, K) and wq.shape == (N, K) and ws.shape == (N, NGRP)

    nc = _build()

    x16 = x.astype(np.float16)
    in_maps = []
    for c in range(NCORES):
        sl = slice(c * NC_SHARD, (c + 1) * NC_SHARD)
        wq_c = np.ascontiguousarray(wq[sl].T).astype(ml_dtypes.float8_e4m3fn)
        # ws block b rows: concat(ws[:,2b], ws[:,2b+1]) broadcast on 128 rows
        ws_t = ws[sl].T.astype(np.float16)  # [32, 1376]
        ws_rows = ws_t.reshape(NBLK, 2 * NC_SHARD)
        ws_c = np.ascontiguousarray(
            np.broadcast_to(ws_rows[:, None, :], (NBLK, 128, 2 * NC_SHARD))
        ).reshape(NBLK * 128, 2 * NC_SHARD)
        in_maps.append({"x": x16, "wq": wq_c, "ws": ws_c})

    res = run_bass_kernel_spmd(nc, in_maps, core_ids=list(range(NCORES)))
    LAST_RESULTS = res
    out = np.concatenate([r["out"] for r in res.results], axis=1)
    return out


if __name__ == "__main__":
    rng = np.random.default_rng(0)
    xv = rng.standard_normal((M, K)).astype(np.float32)
    wqv = rng.integers(-4, 4, (N, K)).astype(np.int32)
    wsv = (rng.random((N, NGRP)).astype(np.float32) * 0.02 + 1e-4)
    o = kernel(xv, wqv, wsv, GS)
    print("out shape:", o.shape, "finite:", np.isfinite(o).all())


# revision 32
# speedup vs baseline: 1.0178x; 1.0178x over previous
"""Trainium2 Bass kernel for Chn8ActGrp3WgtQuantizedLinear.

Computes: out = fake_quant8_per_row(x) @ dequant(weight_qvals, weight_scales).T

  x:             (1024, 4096)  f32
  weight_qvals:  (11008, 4096) int32, 3-bit values in [-4, 3]
  weight_scales: (11008, 32)   f32, one scale per (out-channel, 128-group)
  out:           (1024, 11008) f32
  group_size:    128

Strategy (tensor parallel over 8 NeuronCores; N=11008 -> 1376/core):
  - host repack (layout/dtype only): x -> fp16; wq -> K-major fp16
    [4096, 1376] (3-bit values exact in fp16); ws -> fp16 pre-broadcast
    [16*128, 2752] (block b = groups 2b/2b+1 on 128 partitions).
  - device per core:
      * dequant W[k,n] = wq * ws_bc on DVE (fp16 2x mode), streamed per
        k-group as the wq/ws DMAs land.
      * activation fake-quant per 128-row m-tile: row min/max via a
        tensor_tensor halving tree (fp16 2x) + one 1x reduce on DVE;
        scale/inv; u = ACT(x*inv + 1536) -> f16 (the output convert
        rounds to integer, RNE, since ulp(1536)=1 for |v|<512); then
        in-place DVE u -= 1536 -> exact integer activations in fp16.
        The clip to [qmin-z, qmax-z] is dropped: without clipping the
        zero-point cancels algebraically (a = round(x/s)); round(x/s)
        escapes the clip range only by 1 lsb on knife-edge row extremes,
        perturbing a handful of elements by one quant step.
      * aT[k, m] via PE transposes (32 per m-tile) staged through fp16
        PSUM tiles (8 groups each) + ACT copies to SBUF.
      * matmul: psum[m=128, n=1376] += aT[:,g,:].T @ W over 32 k-groups
        (512-col psum-bank chunks); m0/m1 group-interleaved so the W
        DMA/dequant ramp is consumed at 2 m-tiles per group; quant for
        m2..m4 pipelined inside the ramp.
      * evict with per-row scale: out = psum * scale[m] (ACT), DMA out.
  - host concatenates the 8 (1024, 1376) shards.
"""

import sys
import types

import ml_dtypes
import numpy as np

M, K, N, GS = 1024, 4096, 11008, 128
NCORES = 8
NC_SHARD = N // NCORES  # 1376
NGRP = K // GS  # 32
NBLK = NGRP // 2  # 16 k-group-pair blocks for the ws stream
MTILES = M // 128  # 8
MAGIC = 1536.0  # 1.5 * 2**10: f16 output convert rounds x*inv to int (RNE)

_CACHE = {}
LAST_RESULTS = None


def _install_axon_ntff_hook():
    """Register the NTFF profile hook if the container's antenv lacks it.

    Only needed for trace=True (BASS_TRACE=1); degrades silently."""
    try:
        if "antenv.axon_hooks" in sys.modules:
            return
        import antenv

        mod = types.ModuleType("antenv.axon_hooks")
        _state = {"hook": None}
        mod.set_axon_ntff_profile_hook = lambda h: _state.__setitem__("hook", h)
        mod.get_axon_ntff_profile_hook = lambda: _state["hook"]
        sys.modules["antenv.axon_hooks"] = mod
        antenv.axon_hooks = mod

        from trn_agent_boot.trn_boot import _ntff_profile_via_ctypes

        mod.set_axon_ntff_profile_hook(
            _ntff_profile_via_ctypes("/opt/axon/libaxon_pjrt.so")
        )
    except Exception:
        pass


def _build():
    if "nc" in _CACHE:
        return _CACHE["nc"]

    import contextlib

    import concourse.tile as tile
    from concourse import bacc, mybir
    from concourse.masks import make_identity

    dt = mybir.dt
    F32, F16 = dt.float32, dt.float16
    ALU = mybir.AluOpType
    ACTF = mybir.ActivationFunctionType
    AX = mybir.AxisListType

    nc = bacc.Bacc("TRN2", target_bir_lowering=False, debug=False,
                   num_devices=NCORES)

    x_d = nc.dram_tensor("x", [M, K], F16, kind="ExternalInput").ap()
    wq_d = nc.dram_tensor("wq", [K, NC_SHARD], dt.float8e4,
                          kind="ExternalInput").ap()
    ws_d = nc.dram_tensor("ws", [NBLK * 128, 2 * NC_SHARD], F16,
                          kind="ExternalInput").ap()
    out_d = nc.dram_tensor("out", [M, NC_SHARD], F32, kind="ExternalOutput").ap()

    CHUNKS = [(c, min(512, NC_SHARD - c)) for c in range(0, NC_SHARD, 512)]

    with tile.TileContext(nc) as tc:
        ctx = contextlib.ExitStack()
        with ctx:
            consts = ctx.enter_context(tc.tile_pool(name="consts", bufs=1))
            wpool = ctx.enter_context(tc.tile_pool(name="w", bufs=1))
            wqld = ctx.enter_context(tc.tile_pool(name="wqld", bufs=4))
            wsb = ctx.enter_context(tc.tile_pool(name="ws", bufs=3))
            xp = ctx.enter_context(tc.tile_pool(name="x", bufs=3))
            up = ctx.enter_context(tc.tile_pool(name="u", bufs=3))
            atp = ctx.enter_context(tc.tile_pool(name="at", bufs=4))
            tre = ctx.enter_context(tc.tile_pool(name="tree", bufs=1))
            outp = ctx.enter_context(tc.tile_pool(name="o", bufs=1))
            vecs = ctx.enter_context(tc.tile_pool(name="v", bufs=8))
            ps_out = ctx.enter_context(
                tc.tile_pool(name="pso", bufs=2, space="PSUM"))
            ps_tr = ctx.enter_context(
                tc.tile_pool(name="pst", bufs=2, space="PSUM"))

            magic_vec = consts.tile([128, 1], F32)
            nc.vector.memset(magic_vec[:], MAGIC)
            ident = consts.tile([128, 128], F16)
            make_identity(nc, ident[:])

            # W holds all dequantized weights, k-major: [k%128, g, n]
            W = wpool.tile([128, NGRP * NC_SHARD], F16)

            x_of = {}
            scale_of = {}
            inv_of = {}
            a_of = {}
            at_of = {}

            def load_x(m, spread=False):
                x_t = xp.tile([128, K], F16, tag="xt")
                if spread:  # startup: first packet on each DMA queue
                    rows = x_d[m * 128:(m + 1) * 128, :]
                    engs = [nc.scalar, nc.sync, nc.gpsimd, nc.scalar]
                    for j, eng in enumerate(engs):
                        sl = slice(j * 1024, (j + 1) * 1024)
                        eng.dma_start(x_t[:, sl], rows[:, sl])
                else:
                    nc.scalar.dma_start(x_t[:], x_d[m * 128:(m + 1) * 128, :])
                x_of[m] = x_t

            def stats(m):
                """DVE row min/max via fp16 tensor_tensor tree + 1x reduce.

                Quarter-granular so each stage gates on one x DMA chunk."""
                x_t = x_of[m]
                mx = vecs.tile([128, 1], F32, tag="mx")
                mn = vecs.tile([128, 1], F32, tag="mn")
                for (op, dst) in ((ALU.max, mx), (ALU.min, mn)):
                    sA = tre.tile([128, 1024], F16, tag="sA")
                    nc.vector.tensor_tensor(sA[:], x_t[:, 0:1024],
                                            x_t[:, 2048:3072], op)
                    sB = tre.tile([128, 1024], F16, tag="sB")
                    nc.vector.tensor_tensor(sB[:], x_t[:, 1024:2048],
                                            x_t[:, 3072:4096], op)
                    sC = tre.tile([128, 1024], F16, tag="sC")
                    nc.vector.tensor_tensor(sC[:], sA[:], sB[:], op)
                    nc.vector.tensor_reduce(dst[:], sC[:], axis=AX.X, op=op)
                xc = vecs.tile([128, 1], F32, tag="xc")
                nc.vector.tensor_scalar(xc[:], mx[:], 0.0, None, ALU.max)
                nn_ = vecs.tile([128, 1], F32, tag="nn")
                nc.vector.tensor_scalar(nn_[:], mn[:], 0.0, None, ALU.min)
                df = vecs.tile([128, 1], F32, tag="df")
                nc.vector.tensor_tensor(df[:], xc[:], nn_[:], ALU.subtract)
                sc = vecs.tile([128, 1], F32, tag="sc")
                nc.vector.tensor_scalar(sc[:], df[:], 1.0 / 255.0, 1e-9,
                                        ALU.mult, ALU.max)
                inv = vecs.tile([128, 1], F32, tag="inv")
                nc.vector.reciprocal(inv[:], sc[:])
                scale_of[m] = sc
                inv_of[m] = inv

            def u_pass(m):
                """ACT: u = f16(x*inv + 1536) — the f16 convert rounds (RNE)."""
                u = up.tile([128, K], F16, tag="u")
                nc.scalar.activation(u[:], x_of[m][:], ACTF.Identity,
                                     bias=magic_vec[:], scale=inv_of[m][:])
                a_of[m] = u

            def a_pass(m):
                """DVE (2x), in place: a = u - 1536 (exact ints in fp16)."""
                u = a_of[m]
                nc.vector.tensor_scalar(u[:], u[:], -MAGIC, None, ALU.add)

            def quant(m):
                stats(m)
                u_pass(m)
                a_pass(m)

            def trT(m):
                """PE transposes via fp16 psum (8 groups per stage) + ACT copy."""
                a_t = a_of[m]
                aT = atp.tile([128, NGRP, 128], F16, tag="aT")
                for q in range(4):
                    st = ps_tr.tile([128, 1024], F16, tag="st")
                    for j in range(8):
                        g = q * 8 + j
                        nc.tensor.transpose(st[:, j * 128:(j + 1) * 128],
                                            a_t[:, g * 128:(g + 1) * 128],
                                            ident[:])
                    nc.scalar.copy(aT[:, q * 8:(q + 1) * 8, :]
                                   .rearrange("p g m -> p (g m)"), st[:])
                at_of[m] = aT

            def ws_block(b):
                ws_bc = wsb.tile([128, 2 * NC_SHARD], F16, tag="wsb")
                nc.gpsimd.dma_start(ws_bc[:], ws_d[b * 128:(b + 1) * 128, :])
                return ws_bc

            def wq_deq(g, ws_bc, j, dma=None):
                wq_t = wqld.tile([128, NC_SHARD], dt.float8e4, tag="wq")
                (dma or nc.sync).dma_start(wq_t[:],
                                           wq_d[g * 128:(g + 1) * 128, :])
                # two half-width ops so mm chunks can start on half0 early
                h = NC_SHARD // 2
                for c in (slice(0, h), slice(h, NC_SHARD)):
                    nc.vector.tensor_tensor(
                        W[:, g * NC_SHARD + c.start:g * NC_SHARD + c.stop],
                        wq_t[:, c],
                        ws_bc[:, j * NC_SHARD + c.start:
                              j * NC_SHARD + c.stop], ALU.mult)

            def mm_group(psum, aT, g):
                for (c0, cw) in CHUNKS:
                    nc.tensor.matmul(psum[:, c0:c0 + cw],
                                     lhsT=aT[:, g, :],
                                     rhs=W[:, g * NC_SHARD + c0:
                                           g * NC_SHARD + c0 + cw],
                                     start=(g == 0), stop=(g == NGRP - 1))

            def evict(m, psum):
                o_t = outp.tile([128, NC_SHARD], F32, tag="o")
                nc.scalar.activation(o_t[:], psum[:], ACTF.Identity,
                                     bias=0.0, scale=scale_of[m][:])
                nc.sync.dma_start(out_d[m * 128:(m + 1) * 128, :], o_t[:])

            # ---- emission ----
            load_x(0, spread=True)
            load_x(1)
            quant(0)
            quant(1)
            trT(0)
            trT(1)

            # Fused W stream + m0/m1 matmul ramp: each block's 2 groups are
            # dequantized and immediately consumed by both m-tiles, so the
            # ramp is paced by the wq/ws DMA streams. Quant for m2..m4 is
            # pipelined into the ramp on the DVE/ACT slack.
            ps0 = ps_out.tile([128, NC_SHARD], F32, tag="psum")
            ps1 = ps_out.tile([128, NC_SHARD], F32, tag="psum")
            for b in range(NBLK):
                ws_bc = ws_block(b)
                for j in range(2):
                    g = 2 * b + j
                    # block 0's wq rides the scalar queue behind x0/x1, so
                    # its dequant can't preempt the m0/m1 quant chains on DVE
                    wq_deq(g, ws_bc, j, dma=nc.scalar if b == 0 else None)
                    mm_group(ps0, at_of[0], g)
                    mm_group(ps1, at_of[1], g)
                if b == 2:
                    load_x(2)
                if b == 7:
                    quant(2)
                if b == 12:
                    load_x(3)
                    trT(2)
            evict(0, ps0)
            evict(1, ps1)
            quant(3)

            for m in range(2, MTILES):
                for mf in (m + 2, m + 3):
                    if mf < MTILES and mf not in x_of:
                        load_x(mf)
                if m + 2 < MTILES:
                    quant(m + 2)
                if m + 1 < MTILES:
                    trT(m + 1)
                psum = ps_out.tile([128, NC_SHARD], F32, tag="psum")
                for g in range(NGRP):
                    mm_group(psum, at_of[m], g)
                evict(m, psum)

    nc.compile()
    _CACHE["nc"] = nc
    return nc


def kernel(x, weight_qvals, weight_scales, group_size):
    global LAST_RESULTS
    _install_axon_ntff_hook()
    from concourse.bass_utils import run_bass_kernel_spmd

    x = np.asarray(x, dtype=np.float32)
    wq = np.asarray(weight_qvals)
    ws = np.asarray(weight_scales, dtype=np.float32)
    assert int(group_size) == GS
    assert x.shape == (M, K) and wq.shape == (N, K) and ws.shape == (N, NGRP)

    nc = _build()

    x16 = x.astype(np.float16)
    in_maps = []
    for c in range(NCORES):
        sl = slice(c * NC_SHARD, (c + 1) * NC_SHARD)
        wq_c = np.ascontiguousarray(wq[sl].T).astype(ml_dtypes.float8_e4m3fn)
        # ws block b rows: concat(ws[:,2b], ws[:,2b+1]) broadcast on 128 rows
        ws_t = ws[sl].T.astype(np.float16)  # [32, 1376]
        ws_rows = ws_t.reshape(NBLK, 2 * NC_SHARD)
        ws_c = np.ascontiguousarray(
            np.broadcast_to(ws_rows[:, None, :], (NBLK, 128, 2 * NC_SHARD))
        ).reshape(NBLK * 128, 2 * NC_SHARD)
        in_maps.append({"x": x16, "wq": wq_c, "ws": ws_c})

    res = run_bass_kernel_spmd(nc, in_maps, core_ids=list(range(NCORES)))
    LAST_RESULTS = res
    out = np.concatenate([r["out"] for r in res.results], axis=1)
    return out


if __name__ == "__main__":
    rng = np.random.default_rng(0)
    xv = rng.standard_normal((M, K)).astype(np.float32)
    wqv = rng.integers(-4, 4, (N, K)).astype(np.int32)
    wsv = (rng.random((N, NGRP)).astype(np.float32) * 0.02 + 1e-4)
    o = kernel(xv, wqv, wsv, GS)
    print("out shape:", o.shape, "finite:", np.isfinite(o).all())
